# revision 10
# baseline (speedup 1.0000x reference)
"""Trainium2 Bass kernel for BitShiftMamba (2-layer Mamba + LN + scalar head).

Strategy: pure data-parallel over batch (B=8 -> 1 batch element per core,
zero collectives). Channel-major layout on device: activations stored as
(feature partitions, time free). The selective scan runs as per-(d-chunk, s)
`tensor_tensor_scan` instructions on the Vector engine (fp32 state), with
decays a = exp(delta * A[:, s]) produced by the Scalar engine using the
per-partition `scale` operand of ACTIVATE.

Self-contained: hardcodes all shapes; imports concourse from /opt/trn_rl_repo.
"""
import sys

if "/opt/trn_rl_repo" not in sys.path:
    sys.path.insert(0, "/opt/trn_rl_repo")

import numpy as np

import concourse.bass as bass
import concourse.tile as tile
from concourse import bacc, mybir
from concourse.bass_utils import run_bass_kernel_spmd

F32 = mybir.dt.float32
AF = mybir.ActivationFunctionType
OP = mybir.AluOpType
BF = mybir.dt.bfloat16

# model dims
B, L, DM, DS, DC, NL = 8, 1024, 512, 16, 4, 2
DI = 2 * DM          # 1024
DR = DM // 16        # 32
EPS = 1e-5
NCORES = 8

# kernel tiling
T = 512              # time chunk
NT = L // T          # 2
NDC = DI // 128      # 8 d-chunks
NF = DM // 128       # 4 dm tiles
NFEAT = 2 * DI // 128  # 16 in_proj feature tiles


def _bcast_ap(src: bass.AP, parts: int = 128) -> bass.AP:
    """Partition-broadcast read AP (DRAM source): replicate 1 row to `parts`."""
    return bass.AP(tensor=src.tensor, offset=src.offset,
                   ap=[[0, parts]] + list(src.ap[1:]))


def _build():
    nc = bacc.Bacc("TRN2", target_bir_lowering=False, debug=False,
                   num_devices=NCORES)

    # ---- DRAM I/O ----
    d_rhs2 = nc.dram_tensor("rhs2", [2, L], F32, kind="ExternalInput").ap()
    d_emb = nc.dram_tensor("emb", [2, DM], F32, kind="ExternalInput").ap()
    d_winT = nc.dram_tensor("WinT", [NL, DM, 2 * DI], F32, kind="ExternalInput").ap()
    d_wxT = nc.dram_tensor("WxT", [NL, DI, DR + 2 * DS], F32, kind="ExternalInput").ap()
    d_wdtT = nc.dram_tensor("WdtT", [NL, DR, DI], F32, kind="ExternalInput").ap()
    d_woutT = nc.dram_tensor("WoutT", [NL, DI, DM], F32, kind="ExternalInput").ap()
    d_convw = nc.dram_tensor("convw", [NL, DI, DC], F32, kind="ExternalInput").ap()
    d_convb = nc.dram_tensor("convb", [NL, DI], F32, kind="ExternalInput").ap()
    d_bdt = nc.dram_tensor("bdt", [NL, DI], F32, kind="ExternalInput").ap()
    d_Dp = nc.dram_tensor("Dp", [NL, DI], F32, kind="ExternalInput").ap()
    d_A = nc.dram_tensor("A", [NL, DI, DS], F32, kind="ExternalInput").ap()
    d_gw = nc.dram_tensor("gw", [DM], F32, kind="ExternalInput").ap()
    d_gsum = nc.dram_tensor("gsum", [1, 1], F32, kind="ExternalInput").ap()
    d_cbrow = nc.dram_tensor("cbrow", [1, L], F32, kind="ExternalInput").ap()
    d_logits = nc.dram_tensor("logits", [1, L], F32, kind="ExternalOutput").ap()

    with tile.TileContext(nc) as tc:
        with tc.tile_pool(name="wsmall", bufs=1) as wsmall, \
             tc.tile_pool(name="wwin", bufs=6) as wwin, \
             tc.tile_pool(name="state", bufs=1) as state, \
             tc.tile_pool(name="acts", bufs=1) as acts, \
             tc.tile_pool(name="trans", bufs=2) as trans, \
             tc.tile_pool(name="bcp", bufs=3) as bcp, \
             tc.tile_pool(name="ps", bufs=2, space="PSUM") as ps, \
             tc.tile_pool(name="dramp", bufs=2, space="DRAM") as dramp:

            # ---- persistent small tensors ----
            rhs2 = state.tile([2, L], F32)
            nc.sync.dma_start(out=rhs2[:], in_=d_rhs2[:])
            embt = state.tile([2, DM], F32)
            nc.sync.dma_start(out=embt[:], in_=d_emb[:])
            gwt = state.tile([128, NF], F32)
            nc.sync.dma_start(out=gwt[:], in_=d_gw.rearrange("(f p) -> p f", p=128))
            gsum = state.tile([1, 1], F32)
            nc.sync.dma_start(out=gsum[:], in_=d_gsum[:])
            cbrow = state.tile([1, L], F32)
            nc.sync.dma_start(out=cbrow[:], in_=d_cbrow[:])
            ones_col = state.tile([128, 1], F32)
            nc.vector.memset(ones_col[:], 1.0)
            eps_col = state.tile([1, 1], F32)
            nc.vector.memset(eps_col[:], EPS)

            # per-layer small weights (both layers resident)
            wxs, wdts, convws, convbs, bdts, Dps, As = [], [], [], [], [], [], []
            for li in range(NL):
                wx = wsmall.tile([128, NDC, DR + 2 * DS], F32, tag=f"wx{li}")
                nc.sync.dma_start(out=wx[:], in_=d_wxT[li].rearrange("(c p) m -> p c m", p=128))
                wdt = wsmall.tile([DR, DI], F32, tag=f"wdt{li}")
                nc.sync.dma_start(out=wdt[:], in_=d_wdtT[li])
                cw = wsmall.tile([128, NDC, DC], F32, tag=f"cw{li}")
                nc.sync.dma_start(out=cw[:], in_=d_convw[li].rearrange("(c p) k -> p c k", p=128))
                cb = wsmall.tile([128, NDC], F32, tag=f"cb{li}")
                nc.sync.dma_start(out=cb[:], in_=d_convb[li].rearrange("(c p) -> p c", p=128))
                bd = wsmall.tile([128, NDC], F32, tag=f"bd{li}")
                nc.sync.dma_start(out=bd[:], in_=d_bdt[li].rearrange("(c p) -> p c", p=128))
                dp = wsmall.tile([128, NDC], F32, tag=f"dp{li}")
                nc.sync.dma_start(out=dp[:], in_=d_Dp[li].rearrange("(c p) -> p c", p=128))
                at = wsmall.tile([128, NDC, DS], F32, tag=f"A{li}")
                nc.sync.dma_start(out=at[:], in_=d_A[li].rearrange("(c p) s -> p c s", p=128))
                wxs.append(wx); wdts.append(wdt); convws.append(cw); convbs.append(cb)
                bdts.append(bd); Dps.append(dp); As.append(at)

            # h activations live in DRAM between layers: (128, NF, L) per layer
            h_dram = [dramp.tile([128, NF, L], F32, tag="hl", name=f"hdram{li}")
                      for li in range(NL + 1)]

            # ---- embedding: h = emb.T @ [1-x; x] -> h_dram[0] ----
            for f in range(NF):
                for tcc in range(NT):
                    pse = ps.tile([128, T], F32, tag="mm")
                    nc.tensor.matmul(pse[:], embt[:, f * 128:(f + 1) * 128],
                                     rhs2[:, tcc * T:(tcc + 1) * T],
                                     start=True, stop=True)
                    hst = trans.tile([128, T], F32, tag="hst")
                    nc.scalar.copy(hst[:], pse[:])
                    nc.sync.dma_start(out=h_dram[0][:, f, tcc * T:(tcc + 1) * T],
                                      in_=hst[:])

            # ---- layers ----
            for li in range(NL):
                carry = [state.tile([128, DS], BF, tag=f"carry{c}",
                                    name=f"carry{li}_{c}") for c in range(NDC)]
                z_dram = dramp.tile([128, NDC, L], F32, tag="zd", name=f"zdram{li}")
                tails = [state.tile([128, DC - 1], F32, tag=f"tail{c}",
                                    name=f"tail{li}_{c}") for c in range(NDC)]

                for tcc in range(NT):
                    t0 = tcc * T
                    # ---- in_proj (streamed Win; h from DRAM) ----
                    hin = []
                    for kc in range(NF):
                        hi = acts.tile([128, T], F32, tag=f"hin{kc}",
                                       name=f"hin{li}_{tcc}_{kc}")
                        nc.sync.dma_start(out=hi[:],
                                          in_=h_dram[li][:, kc, t0:t0 + T])
                        hin.append(hi)
                    up_tiles = []
                    for f in range(NFEAT):
                        psx = ps.tile([128, T], F32, tag="mm")
                        for kc in range(NF):
                            wt = wwin.tile([128, 128], F32, tag="win")
                            nc.sync.dma_start(
                                out=wt[:],
                                in_=d_winT[li, kc * 128:(kc + 1) * 128,
                                           f * 128:(f + 1) * 128])
                            nc.tensor.matmul(psx[:], wt[:], hin[kc][:],
                                             start=(kc == 0), stop=(kc == NF - 1))
                        if f < NDC:
                            # save tail of previous chunk BEFORE overwriting slot
                            up = acts.tile([128, T + DC - 1], F32, tag=f"upad{f}",
                                           name=f"up{li}_{tcc}_{f}")
                            if tcc == 0:
                                nc.vector.memset(up[:, 0:DC - 1], 0.0)
                            else:
                                nc.vector.tensor_copy(up[:, 0:DC - 1], tails[f][:])
                            nc.vector.tensor_copy(up[:, DC - 1:T + DC - 1], psx[:])
                            if tcc < NT - 1:
                                nc.vector.tensor_copy(tails[f][:],
                                                      up[:, T:T + DC - 1])
                            up_tiles.append(up)
                        else:
                            zs = trans.tile([128, T], F32, tag="zst")
                            nc.scalar.activation(out=zs[:], in_=psx[:], func=AF.Silu)
                            nc.sync.dma_start(
                                out=z_dram[:, f - NDC, t0:t0 + T], in_=zs[:])

                    # ---- conv + silu ----
                    u_tiles = []
                    for c in range(NDC):
                        cc0 = trans.tile([128, T], F32, tag="cc0", bufs=1)
                        nc.vector.tensor_scalar_mul(cc0[:], up_tiles[c][:, 0:T],
                                                    convws[li][:, c, 0:1])
                        cc1 = trans.tile([128, T], F32, tag="cc1", bufs=1)
                        nc.vector.scalar_tensor_tensor(
                            out=cc1[:], in0=up_tiles[c][:, 1:1 + T],
                            scalar=convws[li][:, c, 1:2], in1=cc0[:],
                            op0=OP.mult, op1=OP.add)
                        cc2 = trans.tile([128, T], F32, tag="cc0", bufs=1,
                                         name=f"cc2_{li}_{tcc}_{c}")
                        nc.vector.scalar_tensor_tensor(
                            out=cc2[:], in0=up_tiles[c][:, 2:2 + T],
                            scalar=convws[li][:, c, 2:3], in1=cc1[:],
                            op0=OP.mult, op1=OP.add)
                        cc3 = trans.tile([128, T], F32, tag="cc1", bufs=1,
                                         name=f"cc3_{li}_{tcc}_{c}")
                        nc.vector.scalar_tensor_tensor(
                            out=cc3[:], in0=up_tiles[c][:, 3:3 + T],
                            scalar=convws[li][:, c, 3:4], in1=cc2[:],
                            op0=OP.mult, op1=OP.add)
                        ut = acts.tile([128, T], F32, tag=f"u{c}",
                                       name=f"u{li}_{tcc}_{c}")
                        nc.scalar.activation(out=ut[:], in_=cc3[:], func=AF.Silu,
                                             bias=convbs[li][:, c:c + 1])
                        u_tiles.append(ut)

                    # ---- x_proj ----
                    xd = ps.tile([DR + 2 * DS, T], F32, tag="xd", bufs=1)
                    for c in range(NDC):
                        nc.tensor.matmul(xd[:], wxs[li][:, c, :], u_tiles[c][:],
                                         start=(c == 0), stop=(c == NDC - 1))
                    dt_sb = acts.tile([DR, T], F32, tag="dtsb")
                    nc.vector.tensor_copy(dt_sb[:], xd[0:DR, :])
                    bc_sb = acts.tile([2 * DS, T], BF, tag="bcsb")
                    nc.vector.tensor_copy(bc_sb[:], xd[DR:DR + 2 * DS, :])
                    bcd = dramp.tile([2 * DS, T], BF, tag="bc")
                    nc.sync.dma_start(out=bcd[:], in_=bc_sb[:])

                    # ---- delta = softplus(Wdt @ dt + bdt), v = delta*u ----
                    d_tiles, v_tiles = [], []
                    for c in range(NDC):
                        psd = ps.tile([128, T], F32, tag="mm")
                        nc.tensor.matmul(psd[:], wdts[li][:, c * 128:(c + 1) * 128],
                                         dt_sb[:], start=True, stop=True)
                        ed = trans.tile([128, T], F32, tag="ed", bufs=1)
                        nc.scalar.activation(out=ed[:], in_=psd[:], func=AF.Exp,
                                             bias=bdts[li][:, c:c + 1])
                        dl = acts.tile([128, T], F32, tag=f"delta{c}",
                                       name=f"delta{li}_{tcc}_{c}")
                        nc.scalar.activation(out=dl[:], in_=ed[:], func=AF.Ln, bias=1.0)
                        d_tiles.append(dl)
                        vt = acts.tile([128, T], BF, tag=f"v{c}",
                                       name=f"v{li}_{tcc}_{c}")
                        nc.vector.tensor_mul(vt[:], dl[:], u_tiles[c][:])
                        v_tiles.append(vt)

                    # ---- selective scan over states s ----
                    y_tiles = [None] * NDC
                    for s in range(DS):
                        bb = bcp.tile([128, T], BF, tag="Bb")
                        nc.sync.dma_start(out=bb[:], in_=_bcast_ap(bcd[s:s + 1, :]))
                        cbt = bcp.tile([128, T], BF, tag="Cb")
                        nc.sync.dma_start(out=cbt[:],
                                          in_=_bcast_ap(bcd[DS + s:DS + s + 1, :]))
                        for c in range(NDC):
                            eng = nc.gpsimd if c >= 6 else nc.vector
                            at = trans.tile([128, T], BF, tag="a", bufs=3)
                            nc.scalar.activation(out=at[:], in_=d_tiles[c][:],
                                                 func=AF.Exp,
                                                 scale=As[li][:, c, s:s + 1])
                            xt = trans.tile([128, T], BF, tag="x", bufs=3)
                            eng.tensor_mul(xt[:], v_tiles[c][:], bb[:])
                            ht = trans.tile([128, T], BF, tag="h", bufs=3)
                            init = 0.0 if tcc == 0 else carry[c][:, s:s + 1]
                            nc.vector.tensor_tensor_scan(
                                out=ht[:], data0=at[:], data1=xt[:], initial=init,
                                op0=OP.mult, op1=OP.add)
                            if tcc < NT - 1:
                                eng.tensor_copy(carry[c][:, s:s + 1],
                                                ht[:, T - 1:T])
                            if s == 0:
                                yt = acts.tile([128, T], BF, tag=f"yacc{c}",
                                               name=f"yacc{li}_{tcc}_{c}")
                                eng.tensor_mul(yt[:], ht[:], cbt[:])
                                y_tiles[c] = yt
                            else:
                                mt = trans.tile([128, T], BF, tag="m", bufs=3)
                                eng.tensor_mul(mt[:], ht[:], cbt[:])
                                eng.tensor_add(y_tiles[c][:], mt[:],
                                               y_tiles[c][:])

                    # ---- y = (y + u*Dp) * silu(z); out_proj (c-outer) ----
                    po = [ps.tile([128, T], F32, tag=f"po{f}", bufs=1,
                                  name=f"po{li}_{tcc}_{f}") for f in range(NF)]
                    for c in range(NDC):
                        zr = trans.tile([128, T], F32, tag="zrl")
                        nc.sync.dma_start(out=zr[:],
                                          in_=z_dram[:, c, t0:t0 + T])
                        yf = trans.tile([128, T], F32, tag="yf")
                        nc.vector.scalar_tensor_tensor(
                            out=yf[:], in0=u_tiles[c][:], scalar=Dps[li][:, c:c + 1],
                            in1=y_tiles[c][:], op0=OP.mult, op1=OP.add)
                        yg = trans.tile([128, T], F32, tag="yg")
                        nc.vector.tensor_mul(yg[:], yf[:], zr[:])
                        for f in range(NF):
                            wo = wwin.tile([128, 128], F32, tag="wo", bufs=4)
                            nc.sync.dma_start(
                                out=wo[:],
                                in_=d_woutT[li, c * 128:(c + 1) * 128,
                                            f * 128:(f + 1) * 128])
                            nc.tensor.matmul(po[f][:], wo[:], yg[:],
                                             start=(c == 0), stop=(c == NDC - 1))
                    for f in range(NF):
                        hst = trans.tile([128, T], F32, tag="hst")
                        nc.scalar.copy(hst[:], po[f][:])
                        nc.sync.dma_start(out=h_dram[li + 1][:, f, t0:t0 + T],
                                          in_=hst[:])

            # ---- final layernorm + head (channel reductions via PE) ----
            for tcc in range(NT):
                sl = slice(tcc * T, (tcc + 1) * T)
                s0 = ps.tile([1, T], F32, tag="po0", bufs=1)
                s1 = ps.tile([1, T], F32, tag="po1", bufs=1)
                s2 = ps.tile([1, T], F32, tag="po2", bufs=1)
                hfs = []
                for f in range(NF):
                    hf = trans.tile([128, T], F32, tag="hst")
                    nc.sync.dma_start(out=hf[:], in_=h_dram[NL][:, f, sl])
                    hfs.append(hf)
                for f in range(NF):
                    nc.tensor.matmul(s0[:], ones_col[:], hfs[f][:],
                                     start=(f == 0), stop=(f == NF - 1))
                    nc.tensor.matmul(s1[:], gwt[:, f:f + 1], hfs[f][:],
                                     start=(f == 0), stop=(f == NF - 1))
                    sq = trans.tile([128, T], F32, tag="sq")
                    nc.scalar.activation(out=sq[:], in_=hfs[f][:], func=AF.Square)
                    nc.tensor.matmul(s2[:], ones_col[:], sq[:],
                                     start=(f == 0), stop=(f == NF - 1))
                mu = trans.tile([1, T], F32, tag="hd", bufs=6)
                nc.vector.tensor_scalar_mul(mu[:], s0[:], 1.0 / DM)
                musq = trans.tile([1, T], F32, tag="hd", bufs=6)
                nc.vector.tensor_mul(musq[:], mu[:], mu[:])
                var = trans.tile([1, T], F32, tag="hd", bufs=6)
                nc.vector.scalar_tensor_tensor(out=var[:], in0=s2[:], scalar=1.0 / DM,
                                               in1=musq[:], op0=OP.mult, op1=OP.subtract)
                lnv = trans.tile([1, T], F32, tag="hd", bufs=6)
                nc.scalar.activation(out=lnv[:], in_=var[:], func=AF.Ln,
                                     bias=eps_col[:, 0:1])
                rstd = trans.tile([1, T], F32, tag="hd", bufs=6)
                nc.scalar.activation(out=rstd[:], in_=lnv[:], func=AF.Exp, scale=-0.5)
                # negnum = mu*Gsum - S1 ; logits = Cb - negnum*rstd
                negnum = trans.tile([1, T], F32, tag="hd", bufs=6)
                nc.vector.scalar_tensor_tensor(out=negnum[:], in0=mu[:],
                                               scalar=gsum[:, 0:1], in1=s1[:],
                                               op0=OP.mult, op1=OP.subtract)
                t1 = trans.tile([1, T], F32, tag="hd", bufs=6)
                nc.vector.tensor_mul(t1[:], negnum[:], rstd[:])
                lg = trans.tile([1, T], F32, tag="hd", bufs=6)
                nc.vector.scalar_tensor_tensor(out=lg[:], in0=t1[:], scalar=-1.0,
                                               in1=cbrow[:, sl], op0=OP.mult, op1=OP.add)
                nc.sync.dma_start(out=d_logits[:, sl], in_=lg[:])

    nc.compile()
    return nc


_NC = None
_last_in_maps = None


def kernel(**inputs) -> np.ndarray:
    global _NC, _last_in_maps
    if _NC is None:
        _NC = _build()
    nc = _NC

    x = np.asarray(inputs["x"])
    emb = np.asarray(inputs["emb"], np.float32)
    Win = np.asarray(inputs["Win"], np.float32)
    conv_w = np.asarray(inputs["conv_w"], np.float32)
    conv_b = np.asarray(inputs["conv_b"], np.float32)
    Wx = np.asarray(inputs["Wx"], np.float32)
    Wdt = np.asarray(inputs["Wdt"], np.float32)
    bdt = np.asarray(inputs["bdt"], np.float32)
    A_log = np.asarray(inputs["A_log"], np.float32)
    Dp = np.asarray(inputs["Dp"], np.float32)
    Wout = np.asarray(inputs["Wout"], np.float32)
    ln_g = np.asarray(inputs["ln_g"], np.float32)
    ln_b = np.asarray(inputs["ln_b"], np.float32)
    head_w = np.asarray(inputs["head_w"], np.float32)
    head_b = np.asarray(inputs["head_b"], np.float32)

    winT = np.ascontiguousarray(np.swapaxes(Win, 1, 2))      # (NL, DM, 2DI)
    wxT = np.ascontiguousarray(np.swapaxes(Wx, 1, 2))        # (NL, DI, DR+2DS)
    wdtT = np.ascontiguousarray(np.swapaxes(Wdt, 1, 2))      # (NL, DR, DI)
    woutT = np.ascontiguousarray(np.swapaxes(Wout, 1, 2))    # (NL, DI, DM)
    A = (-np.exp(A_log)).astype(np.float32)                  # (NL, DI, DS)
    gw = (ln_g * head_w).astype(np.float32)
    gsum = np.full((1, 1), gw.sum(), np.float32)
    cb = float((ln_b * head_w).sum() + head_b)
    cbrow = np.full((1, L), cb, np.float32)

    shared = {
        "emb": emb, "WinT": winT, "WxT": wxT, "WdtT": wdtT, "WoutT": woutT,
        "convw": conv_w, "convb": conv_b, "bdt": bdt, "Dp": Dp, "A": A,
        "gw": gw, "gsum": gsum, "cbrow": cbrow,
    }
    in_maps = []
    for bi in range(NCORES):
        xb = x[bi].astype(np.float32)
        rhs2 = np.stack([1.0 - xb, xb]).astype(np.float32)   # (2, L)
        m = dict(shared)
        m["rhs2"] = rhs2
        in_maps.append(m)

    _last_in_maps = in_maps
    res = run_bass_kernel_spmd(nc, in_maps, list(range(NCORES)))
    out = np.stack([res.results[i]["logits"][0] for i in range(NCORES)])
    return out.astype(np.float32)


# revision 11
# speedup vs baseline: 1.1177x; 1.1177x over previous
"""Trainium2 Bass kernel for BitShiftMamba (2-layer Mamba + LN + scalar head).

Strategy: pure data-parallel over batch (B=8 -> 1 batch element per core,
zero collectives). Channel-major layout on device: activations stored as
(feature partitions, time free). The selective scan runs as per-(d-chunk, s)
`tensor_tensor_scan` instructions on the Vector engine (fp32 state), with
decays a = exp(delta * A[:, s]) produced by the Scalar engine using the
per-partition `scale` operand of ACTIVATE.

Self-contained: hardcodes all shapes; imports concourse from /opt/trn_rl_repo.
"""
import sys

if "/opt/trn_rl_repo" not in sys.path:
    sys.path.insert(0, "/opt/trn_rl_repo")

import numpy as np

import concourse.bass as bass
import concourse.tile as tile
from concourse import bacc, mybir
from concourse.bass_utils import run_bass_kernel_spmd

F32 = mybir.dt.float32
AF = mybir.ActivationFunctionType
OP = mybir.AluOpType
BF = mybir.dt.bfloat16

# model dims
B, L, DM, DS, DC, NL = 8, 1024, 512, 16, 4, 2
DI = 2 * DM          # 1024
DR = DM // 16        # 32
EPS = 1e-5
NCORES = 8

# kernel tiling
T = 512              # time chunk
NT = L // T          # 2
NDC = DI // 128      # 8 d-chunks
NF = DM // 128       # 4 dm tiles
NFEAT = 2 * DI // 128  # 16 in_proj feature tiles


def _bcast_ap(src: bass.AP, parts: int = 128) -> bass.AP:
    """Partition-broadcast read AP (DRAM source): replicate 1 row to `parts`."""
    return bass.AP(tensor=src.tensor, offset=src.offset,
                   ap=[[0, parts]] + list(src.ap[1:]))


def _build():
    nc = bacc.Bacc("TRN2", target_bir_lowering=False, debug=False,
                   num_devices=NCORES)

    # ---- DRAM I/O ----
    d_rhs2 = nc.dram_tensor("rhs2", [2, L], F32, kind="ExternalInput").ap()
    d_emb = nc.dram_tensor("emb", [2, DM], F32, kind="ExternalInput").ap()
    d_winT = nc.dram_tensor("WinT", [NL, DM, 2 * DI], F32, kind="ExternalInput").ap()
    d_wxT = nc.dram_tensor("WxT", [NL, DI, DR + 2 * DS], F32, kind="ExternalInput").ap()
    d_wdtT = nc.dram_tensor("WdtT", [NL, DR, DI], F32, kind="ExternalInput").ap()
    d_woutT = nc.dram_tensor("WoutT", [NL, DI, DM], F32, kind="ExternalInput").ap()
    d_convw = nc.dram_tensor("convw", [NL, DI, DC], F32, kind="ExternalInput").ap()
    d_convb = nc.dram_tensor("convb", [NL, DI], F32, kind="ExternalInput").ap()
    d_bdt = nc.dram_tensor("bdt", [NL, DI], F32, kind="ExternalInput").ap()
    d_Dp = nc.dram_tensor("Dp", [NL, DI], F32, kind="ExternalInput").ap()
    d_A = nc.dram_tensor("A", [NL, DI, DS], F32, kind="ExternalInput").ap()
    d_gw = nc.dram_tensor("gw", [DM], F32, kind="ExternalInput").ap()
    d_gsum = nc.dram_tensor("gsum", [1, 1], F32, kind="ExternalInput").ap()
    d_cbrow = nc.dram_tensor("cbrow", [1, L], F32, kind="ExternalInput").ap()
    d_logits = nc.dram_tensor("logits", [1, L], F32, kind="ExternalOutput").ap()

    with tile.TileContext(nc) as tc:
        with tc.tile_pool(name="wsmall", bufs=1) as wsmall, \
             tc.tile_pool(name="wwin", bufs=6) as wwin, \
             tc.tile_pool(name="state", bufs=1) as state, \
             tc.tile_pool(name="acts", bufs=1) as acts, \
             tc.tile_pool(name="trans", bufs=2) as trans, \
             tc.tile_pool(name="bcp", bufs=3) as bcp, \
             tc.tile_pool(name="ps", bufs=2, space="PSUM") as ps, \
             tc.tile_pool(name="dramp", bufs=2, space="DRAM") as dramp:

            # ---- persistent small tensors ----
            rhs2 = state.tile([2, L], F32)
            nc.sync.dma_start(out=rhs2[:], in_=d_rhs2[:])
            embt = state.tile([2, DM], F32)
            nc.sync.dma_start(out=embt[:], in_=d_emb[:])
            gwt = state.tile([128, NF], F32)
            nc.sync.dma_start(out=gwt[:], in_=d_gw.rearrange("(f p) -> p f", p=128))
            gsum = state.tile([1, 1], F32)
            nc.sync.dma_start(out=gsum[:], in_=d_gsum[:])
            cbrow = state.tile([1, L], F32)
            nc.sync.dma_start(out=cbrow[:], in_=d_cbrow[:])
            ones_col = state.tile([128, 1], F32)
            nc.vector.memset(ones_col[:], 1.0)
            eps_col = state.tile([1, 1], F32)
            nc.vector.memset(eps_col[:], EPS)

            # per-layer small weights (both layers resident)
            wxs, wdts, convws, convbs, bdts, Dps, As = [], [], [], [], [], [], []
            for li in range(NL):
                wx = wsmall.tile([128, NDC, DR + 2 * DS], F32, tag=f"wx{li}")
                nc.sync.dma_start(out=wx[:], in_=d_wxT[li].rearrange("(c p) m -> p c m", p=128))
                wdt = wsmall.tile([DR, DI], F32, tag=f"wdt{li}")
                nc.sync.dma_start(out=wdt[:], in_=d_wdtT[li])
                cw = wsmall.tile([128, NDC, DC], F32, tag=f"cw{li}")
                nc.sync.dma_start(out=cw[:], in_=d_convw[li].rearrange("(c p) k -> p c k", p=128))
                cb = wsmall.tile([128, NDC], F32, tag=f"cb{li}")
                nc.sync.dma_start(out=cb[:], in_=d_convb[li].rearrange("(c p) -> p c", p=128))
                bd = wsmall.tile([128, NDC], F32, tag=f"bd{li}")
                nc.sync.dma_start(out=bd[:], in_=d_bdt[li].rearrange("(c p) -> p c", p=128))
                dp = wsmall.tile([128, NDC], F32, tag=f"dp{li}")
                nc.sync.dma_start(out=dp[:], in_=d_Dp[li].rearrange("(c p) -> p c", p=128))
                at = wsmall.tile([128, NDC, DS], F32, tag=f"A{li}")
                nc.sync.dma_start(out=at[:], in_=d_A[li].rearrange("(c p) s -> p c s", p=128))
                wxs.append(wx); wdts.append(wdt); convws.append(cw); convbs.append(cb)
                bdts.append(bd); Dps.append(dp); As.append(at)

            # h activations live in DRAM between layers: (128, NF, L) per layer
            h_dram = [dramp.tile([128, NF, L], F32, tag="hl", name=f"hdram{li}")
                      for li in range(NL + 1)]

            # ---- embedding: h = emb.T @ [1-x; x] -> h_dram[0] ----
            for f in range(NF):
                for tcc in range(NT):
                    pse = ps.tile([128, T], F32, tag="mm")
                    nc.tensor.matmul(pse[:], embt[:, f * 128:(f + 1) * 128],
                                     rhs2[:, tcc * T:(tcc + 1) * T],
                                     start=True, stop=True)
                    hst = trans.tile([128, T], F32, tag="hst")
                    nc.scalar.copy(hst[:], pse[:])
                    nc.sync.dma_start(out=h_dram[0][:, f, tcc * T:(tcc + 1) * T],
                                      in_=hst[:])

            # ---- layers ----
            for li in range(NL):
                carry = [state.tile([128, DS], BF, tag=f"carry{c}",
                                    name=f"carry{li}_{c}") for c in range(NDC)]
                z_dram = dramp.tile([128, NDC, L], F32, tag="zd", name=f"zdram{li}")
                tails = [state.tile([128, DC - 1], F32, tag=f"tail{c}",
                                    name=f"tail{li}_{c}") for c in range(NDC)]

                for tcc in range(NT):
                    t0 = tcc * T
                    # ---- in_proj (streamed Win; h from DRAM) ----
                    hin = []
                    for kc in range(NF):
                        hi = acts.tile([128, T], F32, tag=f"hin{kc}",
                                       name=f"hin{li}_{tcc}_{kc}")
                        nc.sync.dma_start(out=hi[:],
                                          in_=h_dram[li][:, kc, t0:t0 + T])
                        hin.append(hi)
                    up_tiles = []
                    for f in range(NFEAT):
                        psx = ps.tile([128, T], F32, tag="mm")
                        for kc in range(NF):
                            wt = wwin.tile([128, 128], F32, tag="win")
                            nc.sync.dma_start(
                                out=wt[:],
                                in_=d_winT[li, kc * 128:(kc + 1) * 128,
                                           f * 128:(f + 1) * 128])
                            nc.tensor.matmul(psx[:], wt[:], hin[kc][:],
                                             start=(kc == 0), stop=(kc == NF - 1))
                        if f < NDC:
                            # save tail of previous chunk BEFORE overwriting slot
                            up = acts.tile([128, T + DC - 1], F32, tag=f"upad{f}",
                                           name=f"up{li}_{tcc}_{f}")
                            if tcc == 0:
                                nc.vector.memset(up[:, 0:DC - 1], 0.0)
                            else:
                                nc.vector.tensor_copy(up[:, 0:DC - 1], tails[f][:])
                            nc.vector.tensor_copy(up[:, DC - 1:T + DC - 1], psx[:])
                            if tcc < NT - 1:
                                nc.vector.tensor_copy(tails[f][:],
                                                      up[:, T:T + DC - 1])
                            up_tiles.append(up)
                        else:
                            zs = trans.tile([128, T], F32, tag="zst")
                            nc.scalar.activation(out=zs[:], in_=psx[:], func=AF.Silu)
                            nc.sync.dma_start(
                                out=z_dram[:, f - NDC, t0:t0 + T], in_=zs[:])

                    # ---- conv + silu ----
                    u_tiles = []
                    for c in range(NDC):
                        cc0 = trans.tile([128, T], F32, tag="cc0", bufs=1)
                        nc.vector.tensor_scalar_mul(cc0[:], up_tiles[c][:, 0:T],
                                                    convws[li][:, c, 0:1])
                        cc1 = trans.tile([128, T], F32, tag="cc1", bufs=1)
                        nc.vector.scalar_tensor_tensor(
                            out=cc1[:], in0=up_tiles[c][:, 1:1 + T],
                            scalar=convws[li][:, c, 1:2], in1=cc0[:],
                            op0=OP.mult, op1=OP.add)
                        cc2 = trans.tile([128, T], F32, tag="cc0", bufs=1,
                                         name=f"cc2_{li}_{tcc}_{c}")
                        nc.vector.scalar_tensor_tensor(
                            out=cc2[:], in0=up_tiles[c][:, 2:2 + T],
                            scalar=convws[li][:, c, 2:3], in1=cc1[:],
                            op0=OP.mult, op1=OP.add)
                        cc3 = trans.tile([128, T], F32, tag="cc1", bufs=1,
                                         name=f"cc3_{li}_{tcc}_{c}")
                        nc.vector.scalar_tensor_tensor(
                            out=cc3[:], in0=up_tiles[c][:, 3:3 + T],
                            scalar=convws[li][:, c, 3:4], in1=cc2[:],
                            op0=OP.mult, op1=OP.add)
                        ut = acts.tile([128, T], F32, tag=f"u{c}",
                                       name=f"u{li}_{tcc}_{c}")
                        nc.scalar.activation(out=ut[:], in_=cc3[:], func=AF.Silu,
                                             bias=convbs[li][:, c:c + 1])
                        u_tiles.append(ut)

                    # ---- x_proj ----
                    xd = ps.tile([DR + 2 * DS, T], F32, tag="xd", bufs=1)
                    for c in range(NDC):
                        nc.tensor.matmul(xd[:], wxs[li][:, c, :], u_tiles[c][:],
                                         start=(c == 0), stop=(c == NDC - 1))
                    dt_sb = acts.tile([DR, T], F32, tag="dtsb")
                    nc.vector.tensor_copy(dt_sb[:], xd[0:DR, :])
                    bc_sb = acts.tile([2 * DS, T], BF, tag="bcsb")
                    nc.vector.tensor_copy(bc_sb[:], xd[DR:DR + 2 * DS, :])
                    bcd = dramp.tile([2 * DS, T], BF, tag="bc")
                    nc.sync.dma_start(out=bcd[:], in_=bc_sb[:])

                    # ---- delta = softplus(Wdt @ dt + bdt), v = delta*u ----
                    d_tiles, v_tiles = [], []
                    for c in range(NDC):
                        psd = ps.tile([128, T], F32, tag="mm")
                        nc.tensor.matmul(psd[:], wdts[li][:, c * 128:(c + 1) * 128],
                                         dt_sb[:], start=True, stop=True)
                        ed = trans.tile([128, T], F32, tag="ed", bufs=1)
                        nc.scalar.activation(out=ed[:], in_=psd[:], func=AF.Exp,
                                             bias=bdts[li][:, c:c + 1])
                        dl = acts.tile([128, T], F32, tag=f"delta{c}",
                                       name=f"delta{li}_{tcc}_{c}")
                        nc.scalar.activation(out=dl[:], in_=ed[:], func=AF.Ln, bias=1.0)
                        d_tiles.append(dl)
                        vt = acts.tile([128, T], BF, tag=f"v{c}",
                                       name=f"v{li}_{tcc}_{c}")
                        nc.vector.tensor_mul(vt[:], dl[:], u_tiles[c][:])
                        v_tiles.append(vt)

                    # ---- selective scan over states s ----
                    y_tiles = [None] * NDC
                    for s in range(DS):
                        bb = bcp.tile([128, T], BF, tag="Bb")
                        nc.sync.dma_start(out=bb[:], in_=_bcast_ap(bcd[s:s + 1, :]))
                        cbt = bcp.tile([128, T], BF, tag="Cb")
                        nc.sync.dma_start(out=cbt[:],
                                          in_=_bcast_ap(bcd[DS + s:DS + s + 1, :]))
                        for c in range(NDC):
                            eng = nc.vector
                            at = trans.tile([128, T], BF, tag="a", bufs=3)
                            nc.scalar.activation(out=at[:], in_=d_tiles[c][:],
                                                 func=AF.Exp,
                                                 scale=As[li][:, c, s:s + 1])
                            xt = trans.tile([128, T], BF, tag="x", bufs=3)
                            eng.tensor_mul(xt[:], v_tiles[c][:], bb[:])
                            ht = trans.tile([128, T], BF, tag="h", bufs=3)
                            init = 0.0 if tcc == 0 else carry[c][:, s:s + 1]
                            nc.vector.tensor_tensor_scan(
                                out=ht[:], data0=at[:], data1=xt[:], initial=init,
                                op0=OP.mult, op1=OP.add)
                            if tcc < NT - 1:
                                eng.tensor_copy(carry[c][:, s:s + 1],
                                                ht[:, T - 1:T])
                            if s == 0:
                                yt = acts.tile([128, T], BF, tag=f"yacc{c}",
                                               name=f"yacc{li}_{tcc}_{c}")
                                eng.tensor_mul(yt[:], ht[:], cbt[:])
                                y_tiles[c] = yt
                            else:
                                mt = trans.tile([128, T], BF, tag="m", bufs=3)
                                eng.tensor_mul(mt[:], ht[:], cbt[:])
                                eng.tensor_add(y_tiles[c][:], mt[:],
                                               y_tiles[c][:])

                    # ---- y = (y + u*Dp) * silu(z); out_proj (c-outer) ----
                    po = [ps.tile([128, T], F32, tag=f"po{f}", bufs=1,
                                  name=f"po{li}_{tcc}_{f}") for f in range(NF)]
                    for c in range(NDC):
                        zr = trans.tile([128, T], F32, tag="zrl")
                        nc.sync.dma_start(out=zr[:],
                                          in_=z_dram[:, c, t0:t0 + T])
                        yf = trans.tile([128, T], F32, tag="yf")
                        nc.vector.scalar_tensor_tensor(
                            out=yf[:], in0=u_tiles[c][:], scalar=Dps[li][:, c:c + 1],
                            in1=y_tiles[c][:], op0=OP.mult, op1=OP.add)
                        yg = trans.tile([128, T], F32, tag="yg")
                        nc.vector.tensor_mul(yg[:], yf[:], zr[:])
                        for f in range(NF):
                            wo = wwin.tile([128, 128], F32, tag="wo", bufs=4)
                            nc.sync.dma_start(
                                out=wo[:],
                                in_=d_woutT[li, c * 128:(c + 1) * 128,
                                            f * 128:(f + 1) * 128])
                            nc.tensor.matmul(po[f][:], wo[:], yg[:],
                                             start=(c == 0), stop=(c == NDC - 1))
                    for f in range(NF):
                        hst = trans.tile([128, T], F32, tag="hst")
                        nc.scalar.copy(hst[:], po[f][:])
                        nc.sync.dma_start(out=h_dram[li + 1][:, f, t0:t0 + T],
                                          in_=hst[:])

            # ---- final layernorm + head (channel reductions via PE) ----
            for tcc in range(NT):
                sl = slice(tcc * T, (tcc + 1) * T)
                s0 = ps.tile([1, T], F32, tag="po0", bufs=1)
                s1 = ps.tile([1, T], F32, tag="po1", bufs=1)
                s2 = ps.tile([1, T], F32, tag="po2", bufs=1)
                hfs = []
                for f in range(NF):
                    hf = trans.tile([128, T], F32, tag="hst")
                    nc.sync.dma_start(out=hf[:], in_=h_dram[NL][:, f, sl])
                    hfs.append(hf)
                for f in range(NF):
                    nc.tensor.matmul(s0[:], ones_col[:], hfs[f][:],
                                     start=(f == 0), stop=(f == NF - 1))
                    nc.tensor.matmul(s1[:], gwt[:, f:f + 1], hfs[f][:],
                                     start=(f == 0), stop=(f == NF - 1))
                    sq = trans.tile([128, T], F32, tag="sq")
                    nc.scalar.activation(out=sq[:], in_=hfs[f][:], func=AF.Square)
                    nc.tensor.matmul(s2[:], ones_col[:], sq[:],
                                     start=(f == 0), stop=(f == NF - 1))
                mu = trans.tile([1, T], F32, tag="hd", bufs=6)
                nc.vector.tensor_scalar_mul(mu[:], s0[:], 1.0 / DM)
                musq = trans.tile([1, T], F32, tag="hd", bufs=6)
                nc.vector.tensor_mul(musq[:], mu[:], mu[:])
                var = trans.tile([1, T], F32, tag="hd", bufs=6)
                nc.vector.scalar_tensor_tensor(out=var[:], in0=s2[:], scalar=1.0 / DM,
                                               in1=musq[:], op0=OP.mult, op1=OP.subtract)
                lnv = trans.tile([1, T], F32, tag="hd", bufs=6)
                nc.scalar.activation(out=lnv[:], in_=var[:], func=AF.Ln,
                                     bias=eps_col[:, 0:1])
                rstd = trans.tile([1, T], F32, tag="hd", bufs=6)
                nc.scalar.activation(out=rstd[:], in_=lnv[:], func=AF.Exp, scale=-0.5)
                # negnum = mu*Gsum - S1 ; logits = Cb - negnum*rstd
                negnum = trans.tile([1, T], F32, tag="hd", bufs=6)
                nc.vector.scalar_tensor_tensor(out=negnum[:], in0=mu[:],
                                               scalar=gsum[:, 0:1], in1=s1[:],
                                               op0=OP.mult, op1=OP.subtract)
                t1 = trans.tile([1, T], F32, tag="hd", bufs=6)
                nc.vector.tensor_mul(t1[:], negnum[:], rstd[:])
                lg = trans.tile([1, T], F32, tag="hd", bufs=6)
                nc.vector.scalar_tensor_tensor(out=lg[:], in0=t1[:], scalar=-1.0,
                                               in1=cbrow[:, sl], op0=OP.mult, op1=OP.add)
                nc.sync.dma_start(out=d_logits[:, sl], in_=lg[:])

    nc.compile()
    return nc


_NC = None
_last_in_maps = None


def kernel(**inputs) -> np.ndarray:
    global _NC, _last_in_maps
    if _NC is None:
        _NC = _build()
    nc = _NC

    x = np.asarray(inputs["x"])
    emb = np.asarray(inputs["emb"], np.float32)
    Win = np.asarray(inputs["Win"], np.float32)
    conv_w = np.asarray(inputs["conv_w"], np.float32)
    conv_b = np.asarray(inputs["conv_b"], np.float32)
    Wx = np.asarray(inputs["Wx"], np.float32)
    Wdt = np.asarray(inputs["Wdt"], np.float32)
    bdt = np.asarray(inputs["bdt"], np.float32)
    A_log = np.asarray(inputs["A_log"], np.float32)
    Dp = np.asarray(inputs["Dp"], np.float32)
    Wout = np.asarray(inputs["Wout"], np.float32)
    ln_g = np.asarray(inputs["ln_g"], np.float32)
    ln_b = np.asarray(inputs["ln_b"], np.float32)
    head_w = np.asarray(inputs["head_w"], np.float32)
    head_b = np.asarray(inputs["head_b"], np.float32)

    winT = np.ascontiguousarray(np.swapaxes(Win, 1, 2))      # (NL, DM, 2DI)
    wxT = np.ascontiguousarray(np.swapaxes(Wx, 1, 2))        # (NL, DI, DR+2DS)
    wdtT = np.ascontiguousarray(np.swapaxes(Wdt, 1, 2))      # (NL, DR, DI)
    woutT = np.ascontiguousarray(np.swapaxes(Wout, 1, 2))    # (NL, DI, DM)
    A = (-np.exp(A_log)).astype(np.float32)                  # (NL, DI, DS)
    gw = (ln_g * head_w).astype(np.float32)
    gsum = np.full((1, 1), gw.sum(), np.float32)
    cb = float((ln_b * head_w).sum() + head_b)
    cbrow = np.full((1, L), cb, np.float32)

    shared = {
        "emb": emb, "WinT": winT, "WxT": wxT, "WdtT": wdtT, "WoutT": woutT,
        "convw": conv_w, "convb": conv_b, "bdt": bdt, "Dp": Dp, "A": A,
        "gw": gw, "gsum": gsum, "cbrow": cbrow,
    }
    in_maps = []
    for bi in range(NCORES):
        xb = x[bi].astype(np.float32)
        rhs2 = np.stack([1.0 - xb, xb]).astype(np.float32)   # (2, L)
        m = dict(shared)
        m["rhs2"] = rhs2
        in_maps.append(m)

    _last_in_maps = in_maps
    res = run_bass_kernel_spmd(nc, in_maps, list(range(NCORES)))
    out = np.stack([res.results[i]["logits"][0] for i in range(NCORES)])
    return out.astype(np.float32)
